# revision 13
# baseline (speedup 1.0000x reference)
"""Distributed Trainium2 kernel for nn_Attention_30202210025654.

Data-parallel over batch B=64 across 8 NeuronCores (8 batches/core), with
sync-BN all-reduces of the BatchNorm statistics (sum / sum-of-squares).

Dataflow is feature-major throughout ("features on partitions, tokens on the
free axis"), which makes the two train-mode BatchNorms free-axis reductions
(ACT accum_out / DVE tensor_tensor_reduce) and per-partition affine applies:

  x^T [768,1576] --matmul--> qkv^T [3072,1576] --sync-BN--> attention per
  (head, 2-batch group) --> hswish(out)^T [1536,1576] --matmul--> y^T
  [768,1576] --sync-BN--> output (host transposes back)

Host-side prep (free): weights are pre-transposed, qkv features are permuted
so each head's q and k slices live on identical SBUF partition ranges (the PE
requires lhsT/rhs to share base partitions), and proj_w is pre-divided by 6
to fold the hard-swish /6.

Softmax is computed without max-subtraction (logits are O(5) here, exp is
safe in fp32) in "S^T" orientation [k, q]; the denominator is broadcast
across partitions with an all-ones matmul and inverted as exp(-ln(d)) on the
scalar engine (DVE reciprocal is 8 cyc/elem - too slow).
"""

import numpy as np

import concourse.bass as bass
import concourse.bacc as bacc
import concourse.mybir as mybir
import concourse.tile as tile
from concourse.bass_utils import run_bass_kernel_spmd

F32 = mybir.dt.float32
BF16 = mybir.dt.bfloat16
AX = mybir.AluOpType

# problem shape (hardcoded - kernel.py must be self-contained)
B, N, DIM = 64, 197, 768
H, KD, D = 12, 64, 128
DH = H * D                      # 1536
HQKV = DH + 2 * H * KD          # 3072
EPS = 1e-5
SCALE = KD ** -0.5              # 0.125
NC_CORES = 8
BL = B // NC_CORES              # 8 local batches
TL = BL * N                     # 1576 local tokens
NTOK = B * N                    # 12608 global tokens

NFT = HQKV // 128               # 24 qkv feature tiles (0-5 Q, 6-11 K, 12-23 V)
NKQ = DIM // 128                # 6 contraction tiles for qkv matmul
NFP = DIM // 128                # 6 proj output feature tiles
NKP = DH // 128                 # 12 proj contraction tiles

# token tiles for the 512-wide moving operand
TT = [(0, 512), (512, 512), (1024, 512), (1536, TL - 1536)]

RG = [list(range(NC_CORES))]

import os
DEBUG_PHASE = int(os.environ.get("K_PHASE", "3"))
QKV_SPLIT = int(os.environ.get("K_SPLIT", "0"))   # separate psum tile per t-range
QKV_STATS = int(os.environ.get("K_STATS", "1"))   # accum_out/ttr stats on/off


def _emit(nc):
    """Emit the whole per-core program under a TileContext."""
    xT = nc.dram_tensor("xT", [DIM, TL], F32, kind="ExternalInput")
    wqT = nc.dram_tensor("wqT", [DIM, HQKV], F32, kind="ExternalInput")
    wpT = nc.dram_tensor("wpT", [DH, DIM], F32, kind="ExternalInput")
    gb1 = nc.dram_tensor("gb1", [128, 2 * NFT], F32, kind="ExternalInput")
    gb2 = nc.dram_tensor("gb2", [128, 2 * NFP], F32, kind="ExternalInput")
    ident = nc.dram_tensor("ident", [128, 128], F32, kind="ExternalInput")
    yT = nc.dram_tensor("yT", [DIM, TL], F32, kind="ExternalOutput")
    clso = nc.dram_tensor("cls", [1, BL * 200], F32, kind="ExternalOutput")

    with tile.TileContext(nc) as tc:
        _body(tc, nc, xT, wqT, wpT, gb1, gb2, ident, yT, clso)


def _body(tc, nc, xT, wqT, wpT, gb1, gb2, ident, yT, clso):
    exp = mybir.ActivationFunctionType.Exp
    ln = mybir.ActivationFunctionType.Ln
    copy = mybir.ActivationFunctionType.Copy
    square = mybir.ActivationFunctionType.Square

    with tc.tile_pool(name="const", bufs=1) as cp:
        # constants / small persistent tiles
        ident_bf = cp.tile([128, 128], BF16, tag="identbf")
        ones_bf = cp.tile([128, 128], BF16, tag="onesbf")
        nc.any.memset(ones_bf[:, :], 1.0)
        gb1_sb = cp.tile([128, 2 * NFT], F32, tag="gb1")
        gb2_sb = cp.tile([128, 2 * NFP], F32, tag="gb2")
        nc.sync.dma_start(out=gb1_sb[:, :], in_=gb1[:, :])
        nc.sync.dma_start(out=gb2_sb[:, :], in_=gb2[:, :])
        stats1 = cp.tile([128, 2 * NFT], F32, tag="st1")
        stats1p = cp.tile([128, 4 * NFT], F32, tag="st1p")
        stats1q = cp.tile([128, 4 * NFT], F32, tag="st1q")
        stats2p = cp.tile([128, 4 * NFP], F32, tag="st2p")
        stats2q = cp.tile([128, 4 * NFP], F32, tag="st2q")
        stats1g = cp.tile([128, 2 * NFT], F32, tag="st1g")
        stats2 = cp.tile([128, 2 * NFP], F32, tag="st2")
        stats2g = cp.tile([128, 2 * NFP], F32, tag="st2g")
        acc_lo = cp.tile([128, BL], F32, tag="acclo")
        acc_hi = cp.tile([128, BL], F32, tag="acchi")
        nc.any.memset(acc_lo[:, :], 0.0)
        nc.any.memset(acc_hi[:, :], 0.0)
        # BN scale/shift vectors
        s1 = cp.tile([128, NFT], F32, tag="s1")
        t1 = cp.tile([128, NFT], F32, tag="t1")
        s2 = cp.tile([128, NFP], F32, tag="s2")
        t2 = cp.tile([128, NFP], F32, tag="t2")

        with tc.tile_pool(name="qkv", bufs=1) as qp:
            qkv_sb = [qp.tile([128, TL], BF16, tag=f"qkv{ft}", name=f"qkv{ft}")
                      for ft in range(NFT)]

            # ---------------- phase A: QKV matmul + local stats ----------
            with tc.tile_pool(name="wx", bufs=1) as wxp:
                wqb = [wxp.tile([128, HQKV], BF16, tag=f"wq{k}", name=f"wq{k}")
                       for k in range(NKQ)]
                xb = [wxp.tile([128, TL], BF16, tag=f"x{k}", name=f"xb{k}")
                      for k in range(NKQ)]
                with tc.tile_pool(name="ld", bufs=2) as ldp, \
                     tc.tile_pool(name="psA", bufs=2, space="PSUM") as pqk:
                    # f32 ident -> bf16 (tiny)
                    idf = ldp.tile([128, 128], F32, tag="ldi")
                    nc.sync.dma_start(out=idf[:, :], in_=ident[:, :])
                    nc.any.tensor_copy(ident_bf[:, :], idf[:, :])
                    for k in range(NKQ):
                        wf = ldp.tile([128, HQKV], F32, tag="ld")
                        nc.sync.dma_start(out=wf[:, :], in_=wqT[k * 128:(k + 1) * 128, :])
                        nc.any.tensor_copy(wqb[k][:, :], wf[:, :])
                        xf = ldp.tile([128, TL], F32, tag="ld")
                        nc.sync.dma_start(out=xf[:, :], in_=xT[k * 128:(k + 1) * 128, :])
                        nc.any.tensor_copy(xb[k][:, :], xf[:, :])

                    if DEBUG_PHASE == -1:
                        for f in range(NFP):
                            nc.gpsimd.dma_start(
                                out=yT[f * 128:(f + 1) * 128, :],
                                in_=xb[f][:, :])
                        nc.sync.dma_start(out=clso[0, 0:128],
                                          in_=gb1_sb[0:128, 0:1])
                        return

                    for ft in range(NFT):
                        pss = [pqk.tile([128, tw], F32, tag=f"ps{ti}",
                                        name=f"ps{ft}_{ti}")
                               for ti, (t0, tw) in enumerate(TT)]
                        for ti, (t0, tw) in enumerate(TT):
                            for k in range(NKQ):
                                nc.tensor.matmul(
                                    pss[ti][:, :],
                                    lhsT=wqb[k][:, ft * 128:(ft + 1) * 128],
                                    rhs=xb[k][:, t0:t0 + tw],
                                    start=(k == 0), stop=(k == NKQ - 1),
                                )
                        for ti, (t0, tw) in enumerate(TT):
                            nc.scalar.activation(
                                qkv_sb[ft][:, t0:t0 + tw], pss[ti][:, :], copy,
                                accum_out=stats1p[:, 4 * ft + ti:4 * ft + ti + 1],
                            )
                            sq = ldp.tile([128, 512], BF16, tag="sq",
                                          name=f"sq{ft}_{ti}")
                            nc.scalar.activation(
                                sq[:, 0:tw], pss[ti][:, :], square,
                                accum_out=stats1q[:, 4 * ft + ti:4 * ft + ti + 1],
                            )
                    nc.vector.tensor_reduce(
                        out=stats1[:, 0:NFT],
                        in_=stats1p[:, 0:4 * NFT].rearrange(
                            "p (f t) -> p f t", t=4),
                        op=AX.add, axis=mybir.AxisListType.X,
                    )
                    nc.vector.tensor_reduce(
                        out=stats1[:, NFT:2 * NFT],
                        in_=stats1q[:, 0:4 * NFT].rearrange(
                            "p (f t) -> p f t", t=4),
                        op=AX.add, axis=mybir.AxisListType.X,
                    )

            if DEBUG_PHASE == 0:
                for f in range(NFP):
                    nc.gpsimd.dma_start(out=yT[f * 128:(f + 1) * 128, :],
                                        in_=qkv_sb[f][:, :])
                nc.sync.dma_start(
                    out=clso[0, 0:96],
                    in_=(stats1 if QKV_STATS else gb1_sb)[0:96, 0:1])
                return

            # ---------------- sync-BN 1 ----------------------------------
            with tc.tile_pool(name="dr1", bufs=1, space="DRAM") as dp:
                st_in = dp.tile([128, 2 * NFT], F32, tag="sti")
                st_out = dp.tile([128, 2 * NFT], F32, tag="sto")
                nc.sync.dma_start(out=st_in[:, :], in_=stats1[:, :])
                nc.gpsimd.collective_compute(
                    "AllReduce", AX.add, replica_groups=RG,
                    ins=[st_in[:, :].opt()], outs=[st_out[:, :].opt()],
                )
                nc.sync.dma_start(out=stats1g[:, :], in_=st_out[:, :])

            _bn_vectors(nc, stats1g, gb1_sb, s1, t1, NFT, ln, exp)
            for ft in range(NFT):
                nc.any.tensor_scalar(
                    out=qkv_sb[ft][:, :], in0=qkv_sb[ft][:, :],
                    scalar1=s1[:, ft:ft + 1], scalar2=t1[:, ft:ft + 1],
                    op0=AX.mult, op1=AX.add,
                )

            if DEBUG_PHASE == 1:
                for f in range(NFP):
                    nc.gpsimd.dma_start(out=yT[f * 128:(f + 1) * 128, :],
                                        in_=qkv_sb[f][:, :])
                nc.sync.dma_start(out=clso[0, 0:128],
                                  in_=stats1g[0:128, 0:1])
                return

            # ---------------- phase B: attention --------------------------
            with tc.tile_pool(name="hsw", bufs=1) as hp:
                hsw = [hp.tile([128, TL], BF16, tag=f"h{i}", name=f"hsw{i}")
                       for i in range(NKP)]
                with tc.tile_pool(name="pw", bufs=1) as pwp:
                    wpb = [pwp.tile([128, DIM], BF16, tag=f"wp{k}", name=f"wpb{k}")
                           for k in range(NKP)]
                    with tc.tile_pool(name="ldp2", bufs=2) as ld2:
                        for k in range(NKP):
                            wf = ld2.tile([128, DIM], F32, tag="ld2")
                            nc.sync.dma_start(out=wf[:, :], in_=wpT[k * 128:(k + 1) * 128, :])
                            nc.any.tensor_copy(wpb[k][:, :], wf[:, :])

                    with tc.tile_pool(name="att", bufs=2) as ap, \
                         tc.tile_pool(name="psS", bufs=2, space="PSUM") as pS, \
                         tc.tile_pool(name="psB", bufs=1, space="PSUM") as pB, \
                         tc.tile_pool(name="psO", bufs=2, space="PSUM") as pO:
                        for h in range(H):
                            qt = qkv_sb[h // 2]          # q features
                            kt = qkv_sb[NKQ + h // 2]    # k features
                            vt_src = qkv_sb[2 * NKQ + h]  # v features [128 d, TL]
                            lo = (h % 2) * 64            # partition offset of this head
                            for bg in range(BL // 2):
                                b0 = 2 * bg
                                _attn_group(
                                    tc, nc, h, b0, qt, kt, vt_src, lo,
                                    ones_bf, ident_bf, hsw[h], acc_lo, acc_hi,
                                    ap, pS, pB, pO, exp, ln,
                                )

                    # cls epilogue: mean over heads, then write out columns
                    nc.any.tensor_scalar_mul(acc_lo[:, :], acc_lo[:, :], 1.0 / H)
                    nc.any.tensor_scalar_mul(acc_hi[:, :], acc_hi[:, :], 1.0 / H)
                    for b in range(BL):
                        nc.sync.dma_start(
                            out=clso[0, b * 200:b * 200 + 128],
                            in_=acc_lo[0:128, b:b + 1],
                        )
                        nc.sync.dma_start(
                            out=clso[0, b * 200 + 128:b * 200 + 197],
                            in_=acc_hi[0:69, b:b + 1],
                        )

                    if DEBUG_PHASE == 2:
                        with tc.tile_pool(name="dbg", bufs=2) as dbp:
                            for f in range(NFP):
                                cv = dbp.tile([128, TL], F32, tag="cv",
                                              name=f"dbgcv{f}")
                                nc.any.tensor_copy(cv[:, :], hsw[f][:, :])
                                nc.sync.dma_start(
                                    out=yT[f * 128:(f + 1) * 128, :],
                                    in_=cv[:, :])
                        return

                    # ------------ phase C: proj matmul + sync-BN 2 --------
                    with tc.tile_pool(name="yp", bufs=1) as yp:
                        y_sb = [yp.tile([128, TL], F32, tag=f"y{f}", name=f"ysb{f}")
                                for f in range(NFP)]
                        with tc.tile_pool(name="psY", bufs=2, space="PSUM") as pY, \
                             tc.tile_pool(name="sqy", bufs=2) as sqp:
                            for f in range(NFP):
                                pss = [pY.tile([128, tw], F32, tag=f"py{ti}",
                                               name=f"py{f}_{ti}")
                                       for ti, (t0, tw) in enumerate(TT)]
                                for ti, (t0, tw) in enumerate(TT):
                                    for k in range(NKP):
                                        nc.tensor.matmul(
                                            pss[ti][:, :],
                                            lhsT=wpb[k][:, f * 128:(f + 1) * 128],
                                            rhs=hsw[k][:, t0:t0 + tw],
                                            start=(k == 0), stop=(k == NKP - 1),
                                        )
                                for ti, (t0, tw) in enumerate(TT):
                                    nc.scalar.activation(
                                        y_sb[f][:, t0:t0 + tw], pss[ti][:, :],
                                        copy,
                                        accum_out=stats2p[:, 4 * f + ti:
                                                          4 * f + ti + 1],
                                    )
                                    sq = sqp.tile([128, 512], BF16, tag="sqy",
                                                  name=f"sqy{f}_{ti}")
                                    nc.scalar.activation(
                                        sq[:, 0:tw], pss[ti][:, :], square,
                                        accum_out=stats2q[:, 4 * f + ti:
                                                          4 * f + ti + 1],
                                    )
                            nc.vector.tensor_reduce(
                                out=stats2[:, 0:NFP],
                                in_=stats2p[:, 0:4 * NFP].rearrange(
                                    "p (f t) -> p f t", t=4),
                                op=AX.add, axis=mybir.AxisListType.X,
                            )
                            nc.vector.tensor_reduce(
                                out=stats2[:, NFP:2 * NFP],
                                in_=stats2q[:, 0:4 * NFP].rearrange(
                                    "p (f t) -> p f t", t=4),
                                op=AX.add, axis=mybir.AxisListType.X,
                            )

                        with tc.tile_pool(name="dr2", bufs=1, space="DRAM") as dp:
                            st_in = dp.tile([128, 2 * NFP], F32, tag="st2i")
                            st_out = dp.tile([128, 2 * NFP], F32, tag="st2o")
                            nc.sync.dma_start(out=st_in[:, :], in_=stats2[:, :])
                            nc.gpsimd.collective_compute(
                                "AllReduce", AX.add, replica_groups=RG,
                                ins=[st_in[:, :].opt()], outs=[st_out[:, :].opt()],
                            )
                            nc.sync.dma_start(out=stats2g[:, :], in_=st_out[:, :])

                        _bn_vectors(nc, stats2g, gb2_sb, s2, t2, NFP, ln, exp)
                        for f in range(NFP):
                            nc.any.tensor_scalar(
                                out=y_sb[f][:, :], in0=y_sb[f][:, :],
                                scalar1=s2[:, f:f + 1], scalar2=t2[:, f:f + 1],
                                op0=AX.mult, op1=AX.add,
                            )
                            nc.sync.dma_start(
                                out=yT[f * 128:(f + 1) * 128, :], in_=y_sb[f][:, :],
                            )


def _bn_vectors(nc, stats_g, gb_sb, s, t, nft, ln, exp):
    """s = g*rsqrt(var+eps), t = b - mean*s from all-reduced sum/sumsq.

    rsqrt computed as exp(-0.5*ln(var+eps)) to stay inside the
    natural_log_exp_and_others ACT table set (no sqrt table switch).
    """
    # reuse s/t as scratch: s <- mean for a moment is avoided; use two temps
    # carved out of the stats_g tile footprint instead of extra pools: the
    # ops are [128, nft]-tiny so just overwrite stats_g in place.
    mean = stats_g[:, 0:nft]
    ex2 = stats_g[:, nft:2 * nft]
    nc.any.tensor_scalar_mul(mean, mean, 1.0 / NTOK)
    nc.any.tensor_scalar_mul(ex2, ex2, 1.0 / NTOK)
    # var = ex2 - mean^2  (in place into ex2)
    nc.any.tensor_mul(t[:, 0:nft], mean, mean)
    nc.any.tensor_sub(ex2, ex2, t[:, 0:nft])
    nc.any.tensor_scalar_add(ex2, ex2, EPS)
    nc.scalar.activation(t[:, 0:nft], ex2, ln)
    nc.scalar.activation(s[:, 0:nft], t[:, 0:nft], exp, scale=-0.5)  # rstd
    nc.any.tensor_mul(s[:, 0:nft], gb_sb[:, 0:nft], s[:, 0:nft])
    nc.any.tensor_mul(t[:, 0:nft], mean, s[:, 0:nft])
    nc.any.tensor_sub(t[:, 0:nft], gb_sb[:, nft:2 * nft], t[:, 0:nft])


def _attn_group(tc, nc, h, b0, qt, kt, vt_src, lo, ones_bf, ident_bf, hsw_h,
                acc_lo, acc_hi, ap, pS, pB, pO, exp, ln):
    """Attention for head h, batches (b0, b0+1).

    PSUM S layout [128, 1024]: section (j, kt) at column j*512 + kt*256,
    each 197 wide (padding keeps every matmul inside one 2KB bank).
    j indexes the batch in the pair; kt indexes the 128/69 k-token split.
    """
    S = pS.tile([128, 1024], F32, tag="S")
    for j in range(2):
        b = b0 + j
        c0 = b * N
        # S^T[k, q] = K^T.T @ Q^T for this (b, h)
        nc.tensor.matmul(
            S[0:128, j * 512:j * 512 + N],
            lhsT=kt[lo:lo + 64, c0:c0 + 128],
            rhs=qt[lo:lo + 64, c0:c0 + N],
            start=True, stop=True,
        )
        nc.tensor.matmul(
            S[0:69, j * 512 + 256:j * 512 + 256 + N],
            lhsT=kt[lo:lo + 64, c0 + 128:c0 + N],
            rhs=qt[lo:lo + 64, c0:c0 + N],
            start=True, stop=True,
        )
    E = ap.tile([128, 1024], BF16, tag="E")
    nc.scalar.activation(E[:, :], S[:, :], exp, scale=SCALE)

    # denominator, broadcast over partitions via all-ones stationary
    bc = pB.tile([128, 394], F32, tag="bc")
    for j in range(2):
        nc.tensor.matmul(
            bc[:, j * N:(j + 1) * N],
            lhsT=ones_bf[0:128, 0:128], rhs=E[0:128, j * 512:j * 512 + N],
            start=True, stop=False,
        )
        nc.tensor.matmul(
            bc[:, j * N:(j + 1) * N],
            lhsT=ones_bf[0:69, 0:128], rhs=E[0:69, j * 512 + 256:j * 512 + 256 + N],
            start=False, stop=True,
        )
    # 1/d = exp(-ln d); ln in place on PSUM, exp lands in SBUF
    nc.scalar.activation(bc[:, :], bc[:, :], ln)
    rc = ap.tile([128, 394], BF16, tag="rc")
    nc.scalar.activation(rc[:, :], bc[:, :], exp, scale=-1.0)

    # P = E * (1/d), per section
    P = ap.tile([128, 1024], BF16, tag="P")
    for j in range(2):
        nc.any.tensor_mul(
            P[:, j * 512:j * 512 + N],
            E[:, j * 512:j * 512 + N], rc[:, j * N:(j + 1) * N],
        )
        nc.any.tensor_mul(
            P[0:69, j * 512 + 256:j * 512 + 256 + N],
            E[0:69, j * 512 + 256:j * 512 + 256 + N], rc[0:69, j * N:(j + 1) * N],
        )
        # cls token: column q=0 of P for this batch
        b = b0 + j
        nc.any.tensor_add(
            acc_lo[0:128, b:b + 1], acc_lo[0:128, b:b + 1],
            P[0:128, j * 512:j * 512 + 1],
        )
        nc.any.tensor_add(
            acc_hi[0:69, b:b + 1], acc_hi[0:69, b:b + 1],
            P[0:69, j * 512 + 256:j * 512 + 256 + 1],
        )

    # v^T -> v (token-major) via PE transpose, then PSUM->SBUF copy
    vt = pB.tile([128, 512], BF16, tag="vt")
    for j in range(2):
        b = b0 + j
        c0 = b * N
        nc.tensor.transpose(
            vt[0:128, j * 256:j * 256 + 128], vt_src[:, c0:c0 + 128], ident_bf[:, :],
        )
        nc.tensor.transpose(
            vt[0:69, j * 256 + 128:j * 256 + 256],
            vt_src[:, c0 + 128:c0 + N], ident_bf[:, :],
        )
    vn = ap.tile([128, 512], BF16, tag="vn")
    nc.any.tensor_copy(vn[:, :], vt[:, :])

    # out^T[d, q] = v^T @ P
    ot = pO.tile([128, 394], F32, tag="ot")
    for j in range(2):
        nc.tensor.matmul(
            ot[:, j * N:(j + 1) * N],
            lhsT=vn[0:128, j * 256:j * 256 + 128], rhs=P[0:128, j * 512:j * 512 + N],
            start=True, stop=False,
        )
        nc.tensor.matmul(
            ot[:, j * N:(j + 1) * N],
            lhsT=vn[0:69, j * 256 + 128:j * 256 + 256],
            rhs=P[0:69, j * 512 + 256:j * 512 + 256 + N],
            start=False, stop=True,
        )

    # hswish(x) = x * (relu(x+3) - relu(x-3)) / 6, /6 folded into proj_w
    a = ap.tile([128, 394], F32, tag="ha")
    b2 = ap.tile([128, 394], F32, tag="hb")
    nc.any.tensor_scalar(out=a[:, :], in0=ot[:, :], scalar1=3.0, scalar2=0.0,
                         op0=AX.add, op1=AX.max)
    nc.any.tensor_scalar(out=b2[:, :], in0=ot[:, :], scalar1=-3.0, scalar2=0.0,
                         op0=AX.add, op1=AX.max)
    nc.any.tensor_sub(a[:, :], a[:, :], b2[:, :])
    nc.any.tensor_mul(hsw_h[:, b0 * N:b0 * N + 2 * N], ot[:, :], a[:, :])


# ----------------------------------------------------------------------------
# host side
# ----------------------------------------------------------------------------

_CACHE = {}


def _build():
    if "nc" not in _CACHE:
        nc = bacc.Bacc("TRN2", target_bir_lowering=False, debug=False,
                       num_devices=NC_CORES)
        _emit(nc)
        nc.compile()
        _CACHE["nc"] = nc
    return _CACHE["nc"]


def _feature_perm():
    """Permutation of qkv output features into [Q | K | V] blocks so each
    head's q and k live at the same partition offset of their SBUF tiles."""
    perm = np.empty(HQKV, dtype=np.int64)
    pos = 0
    for blk, width in ((0, KD), (KD, KD), (2 * KD, D)):  # q, k, v offsets
        for h in range(H):
            perm[pos:pos + width] = h * (2 * KD + D) + blk + np.arange(width)
            pos += width
    return perm


def _prep_shared(qkv_w, qkv_g, qkv_b, proj_w, proj_g, proj_b):
    perm = _feature_perm()
    wq = qkv_w[perm]                      # [3072, 768]
    g1 = qkv_g[perm]
    b1 = qkv_b[perm]
    wqT = np.ascontiguousarray(wq.T, dtype=np.float32)          # [768, 3072]
    wpT = np.ascontiguousarray(proj_w.T / 6.0, dtype=np.float32)  # [1536, 768]
    gb1 = np.concatenate(
        [g1.reshape(NFT, 128).T, b1.reshape(NFT, 128).T], axis=1
    ).astype(np.float32)                  # [128, 48]
    gb2 = np.concatenate(
        [proj_g.reshape(NFP, 128).T, proj_b.reshape(NFP, 128).T], axis=1
    ).astype(np.float32)                  # [128, 12]
    ident = np.eye(128, dtype=np.float32)
    return wqT, wpT, gb1, gb2, ident


def run(x, qkv_w, qkv_g, qkv_b, proj_w, proj_g, proj_b, trace=False,
        trace_kwargs=None):
    nc = _build()
    wqT, wpT, gb1, gb2, ident = _prep_shared(
        qkv_w, qkv_g, qkv_b, proj_w, proj_g, proj_b)
    in_maps = []
    for i in range(NC_CORES):
        xs = x[i * BL:(i + 1) * BL].reshape(TL, DIM)
        in_maps.append({
            "xT": np.ascontiguousarray(xs.T, dtype=np.float32),
            "wqT": wqT, "wpT": wpT, "gb1": gb1, "gb2": gb2, "ident": ident,
        })
    kw = {}
    if trace:
        kw["trace"] = True
        if trace_kwargs:
            kw.update(trace_kwargs)
    res = run_bass_kernel_spmd(nc, in_maps, list(range(NC_CORES)), **kw)
    y = np.concatenate(
        [r["yT"].T.reshape(BL, N, DIM) for r in res.results], axis=0)
    cls = np.concatenate(
        [r["cls"].reshape(BL, 200)[:, 1:N] for r in res.results], axis=0)
    return y, cls, res


def kernel(x, qkv_w, qkv_g, qkv_b, proj_w, proj_g, proj_b):
    y, cls, _ = run(x, qkv_w, qkv_g, qkv_b, proj_w, proj_g, proj_b)
    return y, cls


# revision 14
# speedup vs baseline: 1.0718x; 1.0718x over previous
"""Distributed Trainium2 kernel for nn_Attention_30202210025654.

Data-parallel over batch B=64 across 8 NeuronCores (8 batches/core), with
sync-BN all-reduces of the BatchNorm statistics (sum / sum-of-squares).

Dataflow is feature-major throughout ("features on partitions, tokens on the
free axis"), which makes the two train-mode BatchNorms free-axis reductions
(ACT accum_out / DVE tensor_tensor_reduce) and per-partition affine applies:

  x^T [768,1576] --matmul--> qkv^T [3072,1576] --sync-BN--> attention per
  (head, 2-batch group) --> hswish(out)^T [1536,1576] --matmul--> y^T
  [768,1576] --sync-BN--> output (host transposes back)

Host-side prep (free): weights are pre-transposed, qkv features are permuted
so each head's q and k slices live on identical SBUF partition ranges (the PE
requires lhsT/rhs to share base partitions), and proj_w is pre-divided by 6
to fold the hard-swish /6.

Softmax is computed without max-subtraction (logits are O(5) here, exp is
safe in fp32) in "S^T" orientation [k, q]; the denominator is broadcast
across partitions with an all-ones matmul and inverted as exp(-ln(d)) on the
scalar engine (DVE reciprocal is 8 cyc/elem - too slow).
"""

import numpy as np

import concourse.bass as bass
import concourse.bacc as bacc
import concourse.mybir as mybir
import concourse.tile as tile
from concourse.bass_utils import run_bass_kernel_spmd

F32 = mybir.dt.float32
BF16 = mybir.dt.bfloat16
AX = mybir.AluOpType

# problem shape (hardcoded - kernel.py must be self-contained)
B, N, DIM = 64, 197, 768
H, KD, D = 12, 64, 128
DH = H * D                      # 1536
HQKV = DH + 2 * H * KD          # 3072
EPS = 1e-5
SCALE = KD ** -0.5              # 0.125
NC_CORES = 8
BL = B // NC_CORES              # 8 local batches
TL = BL * N                     # 1576 local tokens
NTOK = B * N                    # 12608 global tokens

NFT = HQKV // 128               # 24 qkv feature tiles (0-5 Q, 6-11 K, 12-23 V)
NKQ = DIM // 128                # 6 contraction tiles for qkv matmul
NFP = DIM // 128                # 6 proj output feature tiles
NKP = DH // 128                 # 12 proj contraction tiles

# token tiles for the 512-wide moving operand
TT = [(0, 512), (512, 512), (1024, 512), (1536, TL - 1536)]

RG = [list(range(NC_CORES))]

import os
DEBUG_PHASE = int(os.environ.get("K_PHASE", "3"))
QKV_SPLIT = int(os.environ.get("K_SPLIT", "0"))   # separate psum tile per t-range
QKV_STATS = int(os.environ.get("K_STATS", "1"))   # accum_out/ttr stats on/off


def _emit(nc):
    """Emit the whole per-core program under a TileContext."""
    xT = nc.dram_tensor("xT", [DIM, TL], F32, kind="ExternalInput")
    wqT = nc.dram_tensor("wqT", [DIM, HQKV], F32, kind="ExternalInput")
    wpT = nc.dram_tensor("wpT", [DH, DIM], F32, kind="ExternalInput")
    gb1 = nc.dram_tensor("gb1", [128, 2 * NFT], F32, kind="ExternalInput")
    gb2 = nc.dram_tensor("gb2", [128, 2 * NFP], F32, kind="ExternalInput")
    ident = nc.dram_tensor("ident", [128, 128], F32, kind="ExternalInput")
    yT = nc.dram_tensor("yT", [DIM, TL], F32, kind="ExternalOutput")
    clso = nc.dram_tensor("cls", [1, BL * 200], F32, kind="ExternalOutput")

    with tile.TileContext(nc) as tc:
        _body(tc, nc, xT, wqT, wpT, gb1, gb2, ident, yT, clso)


def _body(tc, nc, xT, wqT, wpT, gb1, gb2, ident, yT, clso):
    exp = mybir.ActivationFunctionType.Exp
    ln = mybir.ActivationFunctionType.Ln
    copy = mybir.ActivationFunctionType.Copy
    square = mybir.ActivationFunctionType.Square

    with tc.tile_pool(name="const", bufs=1) as cp:
        # constants / small persistent tiles
        ident_bf = cp.tile([128, 128], BF16, tag="identbf")
        ones_bf = cp.tile([128, 128], BF16, tag="onesbf")
        nc.any.memset(ones_bf[:, :], 1.0)
        gb1_sb = cp.tile([128, 2 * NFT], F32, tag="gb1")
        gb2_sb = cp.tile([128, 2 * NFP], F32, tag="gb2")
        nc.sync.dma_start(out=gb1_sb[:, :], in_=gb1[:, :])
        nc.sync.dma_start(out=gb2_sb[:, :], in_=gb2[:, :])
        stats1 = cp.tile([128, 2 * NFT], F32, tag="st1")
        stats1p = cp.tile([128, 4 * NFT], F32, tag="st1p")
        stats2p = cp.tile([128, 4 * NFP], F32, tag="st2p")
        stats2q = cp.tile([128, 4 * NFP], F32, tag="st2q")
        stats1g = cp.tile([128, 2 * NFT], F32, tag="st1g")
        stats2 = cp.tile([128, 2 * NFP], F32, tag="st2")
        stats2g = cp.tile([128, 2 * NFP], F32, tag="st2g")
        acc_lo = cp.tile([128, BL], F32, tag="acclo")
        acc_hi = cp.tile([128, BL], F32, tag="acchi")
        nc.any.memset(acc_lo[:, :], 0.0)
        nc.any.memset(acc_hi[:, :], 0.0)
        # BN scale/shift vectors
        s1 = cp.tile([128, NFT], F32, tag="s1")
        t1 = cp.tile([128, NFT], F32, tag="t1")
        s2 = cp.tile([128, NFP], F32, tag="s2")
        t2 = cp.tile([128, NFP], F32, tag="t2")

        with tc.tile_pool(name="qkv", bufs=1) as qp:
            qkv_sb = [qp.tile([128, TL], BF16, tag=f"qkv{ft}", name=f"qkv{ft}")
                      for ft in range(NFT)]

            # ---------------- phase A: QKV matmul + local stats ----------
            with tc.tile_pool(name="wx", bufs=1) as wxp:
                wqb = [wxp.tile([128, HQKV], BF16, tag=f"wq{k}", name=f"wq{k}")
                       for k in range(NKQ)]
                xb = [wxp.tile([128, TL], BF16, tag=f"x{k}", name=f"xb{k}")
                      for k in range(NKQ)]
                with tc.tile_pool(name="ld", bufs=2) as ldp, \
                     tc.tile_pool(name="psA", bufs=2, space="PSUM") as pqk:
                    # f32 ident -> bf16 (tiny)
                    idf = ldp.tile([128, 128], F32, tag="ldi")
                    nc.sync.dma_start(out=idf[:, :], in_=ident[:, :])
                    nc.any.tensor_copy(ident_bf[:, :], idf[:, :])
                    for k in range(NKQ):
                        wf = ldp.tile([128, HQKV], F32, tag="ld")
                        nc.sync.dma_start(out=wf[:, :], in_=wqT[k * 128:(k + 1) * 128, :])
                        nc.any.tensor_copy(wqb[k][:, :], wf[:, :])
                        xf = ldp.tile([128, TL], F32, tag="ld")
                        nc.sync.dma_start(out=xf[:, :], in_=xT[k * 128:(k + 1) * 128, :])
                        nc.any.tensor_copy(xb[k][:, :], xf[:, :])

                    if DEBUG_PHASE == -1:
                        for f in range(NFP):
                            nc.gpsimd.dma_start(
                                out=yT[f * 128:(f + 1) * 128, :],
                                in_=xb[f][:, :])
                        nc.sync.dma_start(out=clso[0, 0:128],
                                          in_=gb1_sb[0:128, 0:1])
                        return

                    for ft in range(NFT):
                        pss = [pqk.tile([128, tw], F32, tag=f"ps{ti}",
                                        name=f"ps{ft}_{ti}")
                               for ti, (t0, tw) in enumerate(TT)]
                        for ti, (t0, tw) in enumerate(TT):
                            for k in range(NKQ):
                                nc.tensor.matmul(
                                    pss[ti][:, :],
                                    lhsT=wqb[k][:, ft * 128:(ft + 1) * 128],
                                    rhs=xb[k][:, t0:t0 + tw],
                                    start=(k == 0), stop=(k == NKQ - 1),
                                )
                        for ti, (t0, tw) in enumerate(TT):
                            nc.scalar.activation(
                                qkv_sb[ft][:, t0:t0 + tw], pss[ti][:, :], copy,
                                accum_out=stats1p[:, 4 * ft + ti:4 * ft + ti + 1],
                            )
                        sq = ldp.tile([128, TL], BF16, tag="sq",
                                      name=f"sq{ft}")
                        nc.vector.tensor_mul(sq[:, :], qkv_sb[ft][:, :],
                                             qkv_sb[ft][:, :])
                        nc.vector.tensor_reduce(
                            out=stats1[:, NFT + ft:NFT + ft + 1],
                            in_=sq[:, :], op=AX.add, axis=mybir.AxisListType.X,
                        )
                    nc.vector.tensor_reduce(
                        out=stats1[:, 0:NFT],
                        in_=stats1p[:, 0:4 * NFT].rearrange(
                            "p (f t) -> p f t", t=4),
                        op=AX.add, axis=mybir.AxisListType.X,
                    )

            if DEBUG_PHASE == 0:
                for f in range(NFP):
                    nc.gpsimd.dma_start(out=yT[f * 128:(f + 1) * 128, :],
                                        in_=qkv_sb[f][:, :])
                nc.sync.dma_start(
                    out=clso[0, 0:96],
                    in_=(stats1 if QKV_STATS else gb1_sb)[0:96, 0:1])
                return

            # ---------------- sync-BN 1 ----------------------------------
            with tc.tile_pool(name="dr1", bufs=1, space="DRAM") as dp:
                st_in = dp.tile([128, 2 * NFT], F32, tag="sti")
                st_out = dp.tile([128, 2 * NFT], F32, tag="sto")
                nc.sync.dma_start(out=st_in[:, :], in_=stats1[:, :])
                nc.gpsimd.collective_compute(
                    "AllReduce", AX.add, replica_groups=RG,
                    ins=[st_in[:, :].opt()], outs=[st_out[:, :].opt()],
                )
                nc.sync.dma_start(out=stats1g[:, :], in_=st_out[:, :])

            _bn_vectors(nc, stats1g, gb1_sb, s1, t1, NFT, ln, exp)
            for ft in range(NFT):
                nc.any.tensor_scalar(
                    out=qkv_sb[ft][:, :], in0=qkv_sb[ft][:, :],
                    scalar1=s1[:, ft:ft + 1], scalar2=t1[:, ft:ft + 1],
                    op0=AX.mult, op1=AX.add,
                )

            if DEBUG_PHASE == 1:
                for f in range(NFP):
                    nc.gpsimd.dma_start(out=yT[f * 128:(f + 1) * 128, :],
                                        in_=qkv_sb[f][:, :])
                nc.sync.dma_start(out=clso[0, 0:128],
                                  in_=stats1g[0:128, 0:1])
                return

            # ---------------- phase B: attention --------------------------
            with tc.tile_pool(name="hsw", bufs=1) as hp:
                hsw = [hp.tile([128, TL], BF16, tag=f"h{i}", name=f"hsw{i}")
                       for i in range(NKP)]
                with tc.tile_pool(name="pw", bufs=1) as pwp:
                    wpb = [pwp.tile([128, DIM], BF16, tag=f"wp{k}", name=f"wpb{k}")
                           for k in range(NKP)]
                    with tc.tile_pool(name="ldp2", bufs=2) as ld2:
                        for k in range(NKP):
                            wf = ld2.tile([128, DIM], F32, tag="ld2")
                            nc.sync.dma_start(out=wf[:, :], in_=wpT[k * 128:(k + 1) * 128, :])
                            nc.any.tensor_copy(wpb[k][:, :], wf[:, :])

                    with tc.tile_pool(name="att", bufs=2) as ap, \
                         tc.tile_pool(name="psS", bufs=2, space="PSUM") as pS, \
                         tc.tile_pool(name="psB", bufs=1, space="PSUM") as pB, \
                         tc.tile_pool(name="psO", bufs=2, space="PSUM") as pO:
                        for h in range(H):
                            qt = qkv_sb[h // 2]          # q features
                            kt = qkv_sb[NKQ + h // 2]    # k features
                            vt_src = qkv_sb[2 * NKQ + h]  # v features [128 d, TL]
                            lo = (h % 2) * 64            # partition offset of this head
                            _attn_head(
                                tc, nc, h, qt, kt, vt_src, lo,
                                ones_bf, ident_bf, hsw[h], acc_lo, acc_hi,
                                ap, pS, pB, pO, exp, ln,
                            )

                    # cls epilogue: mean over heads, then write out columns
                    nc.any.tensor_scalar_mul(acc_lo[:, :], acc_lo[:, :], 1.0 / H)
                    nc.any.tensor_scalar_mul(acc_hi[:, :], acc_hi[:, :], 1.0 / H)
                    for b in range(BL):
                        nc.sync.dma_start(
                            out=clso[0, b * 200:b * 200 + 128],
                            in_=acc_lo[0:128, b:b + 1],
                        )
                        nc.sync.dma_start(
                            out=clso[0, b * 200 + 128:b * 200 + 197],
                            in_=acc_hi[0:69, b:b + 1],
                        )

                    if DEBUG_PHASE == 2:
                        with tc.tile_pool(name="dbg", bufs=2) as dbp:
                            for f in range(NFP):
                                cv = dbp.tile([128, TL], F32, tag="cv",
                                              name=f"dbgcv{f}")
                                nc.any.tensor_copy(cv[:, :], hsw[f][:, :])
                                nc.sync.dma_start(
                                    out=yT[f * 128:(f + 1) * 128, :],
                                    in_=cv[:, :])
                        return

                    # ------------ phase C: proj matmul + sync-BN 2 --------
                    with tc.tile_pool(name="yp", bufs=1) as yp:
                        y_sb = [yp.tile([128, TL], F32, tag=f"y{f}", name=f"ysb{f}")
                                for f in range(NFP)]
                        with tc.tile_pool(name="psY", bufs=2, space="PSUM") as pY, \
                             tc.tile_pool(name="sqy", bufs=2) as sqp:
                            for f in range(NFP):
                                pss = [pY.tile([128, tw], F32, tag=f"py{ti}",
                                               name=f"py{f}_{ti}")
                                       for ti, (t0, tw) in enumerate(TT)]
                                for ti, (t0, tw) in enumerate(TT):
                                    for k in range(NKP):
                                        nc.tensor.matmul(
                                            pss[ti][:, :],
                                            lhsT=wpb[k][:, f * 128:(f + 1) * 128],
                                            rhs=hsw[k][:, t0:t0 + tw],
                                            start=(k == 0), stop=(k == NKP - 1),
                                        )
                                for ti, (t0, tw) in enumerate(TT):
                                    nc.scalar.activation(
                                        y_sb[f][:, t0:t0 + tw], pss[ti][:, :],
                                        copy,
                                        accum_out=stats2p[:, 4 * f + ti:
                                                          4 * f + ti + 1],
                                    )
                                    sq = sqp.tile([128, 512], BF16, tag="sqy",
                                                  name=f"sqy{f}_{ti}")
                                    nc.scalar.activation(
                                        sq[:, 0:tw], pss[ti][:, :], square,
                                        accum_out=stats2q[:, 4 * f + ti:
                                                          4 * f + ti + 1],
                                    )
                            nc.vector.tensor_reduce(
                                out=stats2[:, 0:NFP],
                                in_=stats2p[:, 0:4 * NFP].rearrange(
                                    "p (f t) -> p f t", t=4),
                                op=AX.add, axis=mybir.AxisListType.X,
                            )
                            nc.vector.tensor_reduce(
                                out=stats2[:, NFP:2 * NFP],
                                in_=stats2q[:, 0:4 * NFP].rearrange(
                                    "p (f t) -> p f t", t=4),
                                op=AX.add, axis=mybir.AxisListType.X,
                            )

                        with tc.tile_pool(name="dr2", bufs=1, space="DRAM") as dp:
                            st_in = dp.tile([128, 2 * NFP], F32, tag="st2i")
                            st_out = dp.tile([128, 2 * NFP], F32, tag="st2o")
                            nc.sync.dma_start(out=st_in[:, :], in_=stats2[:, :])
                            nc.gpsimd.collective_compute(
                                "AllReduce", AX.add, replica_groups=RG,
                                ins=[st_in[:, :].opt()], outs=[st_out[:, :].opt()],
                            )
                            nc.sync.dma_start(out=stats2g[:, :], in_=st_out[:, :])

                        _bn_vectors(nc, stats2g, gb2_sb, s2, t2, NFP, ln, exp)
                        for f in range(NFP):
                            nc.any.tensor_scalar(
                                out=y_sb[f][:, :], in0=y_sb[f][:, :],
                                scalar1=s2[:, f:f + 1], scalar2=t2[:, f:f + 1],
                                op0=AX.mult, op1=AX.add,
                            )
                            nc.sync.dma_start(
                                out=yT[f * 128:(f + 1) * 128, :], in_=y_sb[f][:, :],
                            )


def _bn_vectors(nc, stats_g, gb_sb, s, t, nft, ln, exp):
    """s = g*rsqrt(var+eps), t = b - mean*s from all-reduced sum/sumsq.

    rsqrt computed as exp(-0.5*ln(var+eps)) to stay inside the
    natural_log_exp_and_others ACT table set (no sqrt table switch).
    """
    # reuse s/t as scratch: s <- mean for a moment is avoided; use two temps
    # carved out of the stats_g tile footprint instead of extra pools: the
    # ops are [128, nft]-tiny so just overwrite stats_g in place.
    mean = stats_g[:, 0:nft]
    ex2 = stats_g[:, nft:2 * nft]
    nc.any.tensor_scalar_mul(mean, mean, 1.0 / NTOK)
    nc.any.tensor_scalar_mul(ex2, ex2, 1.0 / NTOK)
    # var = ex2 - mean^2  (in place into ex2)
    nc.any.tensor_mul(t[:, 0:nft], mean, mean)
    nc.any.tensor_sub(ex2, ex2, t[:, 0:nft])
    nc.any.tensor_scalar_add(ex2, ex2, EPS)
    nc.scalar.activation(t[:, 0:nft], ex2, ln)
    nc.scalar.activation(s[:, 0:nft], t[:, 0:nft], exp, scale=-0.5)  # rstd
    nc.any.tensor_mul(s[:, 0:nft], gb_sb[:, 0:nft], s[:, 0:nft])
    nc.any.tensor_mul(t[:, 0:nft], mean, s[:, 0:nft])
    nc.any.tensor_sub(t[:, 0:nft], gb_sb[:, nft:2 * nft], t[:, 0:nft])


def _attn_head(tc, nc, h, qt, kt, vt_src, lo, ones_bf, ident_bf, hsw_h,
               acc_lo, acc_hi, ap, pS, pB, pO, exp, ln):
    """Attention for head h, all 8 batches as 4 groups of 2, wave-ordered so
    the scalar engine runs exp(S)x4 -> ln(d)x4 -> exp(-ln d)x4 back to back
    (2 ACT table loads per head instead of 2 per group).

    PSUM S layout [128, 1024]: section (j, kt) at column j*512 + kt*256,
    each 197 wide; padding keeps every matmul inside one 2KB bank.
    PV runs on the UNNORMALIZED exp tile E; the softmax denominator is
    applied to out_raw afterwards (rc is broadcast across partitions by an
    all-ones matmul, inverted as exp(-ln d)).
    """
    NG = BL // 2
    Es, bcs, lnds, rcs = [], [], [], []
    # wave 1: S matmuls + exp per group
    for bg in range(NG):
        b0 = 2 * bg
        S = pS.tile([128, 1024], F32, tag="S", name=f"S{h}_{bg}")
        for j in range(2):
            c0 = (b0 + j) * N
            nc.tensor.matmul(
                S[0:128, j * 512:j * 512 + N],
                lhsT=kt[lo:lo + 64, c0:c0 + 128],
                rhs=qt[lo:lo + 64, c0:c0 + N],
                start=True, stop=True,
            )
            nc.tensor.matmul(
                S[0:69, j * 512 + 256:j * 512 + 256 + N],
                lhsT=kt[lo:lo + 64, c0 + 128:c0 + N],
                rhs=qt[lo:lo + 64, c0:c0 + N],
                start=True, stop=True,
            )
        E = ap.tile([128, 1024], BF16, tag=f"E{bg}", name=f"E{h}_{bg}")
        nc.scalar.activation(E[:, :], S[:, :], exp, scale=SCALE)
        Es.append(E)
    # wave 2: denominator matmuls (broadcast over partitions via all-ones)
    for bg in range(NG):
        E = Es[bg]
        bc = pB.tile([128, 394], F32, tag="bc", name=f"bc{h}_{bg}")
        for j in range(2):
            nc.tensor.matmul(
                bc[:, j * N:(j + 1) * N],
                lhsT=ones_bf[0:128, 0:128], rhs=E[0:128, j * 512:j * 512 + N],
                start=True, stop=False,
            )
            nc.tensor.matmul(
                bc[:, j * N:(j + 1) * N],
                lhsT=ones_bf[0:69, 0:128],
                rhs=E[0:69, j * 512 + 256:j * 512 + 256 + N],
                start=False, stop=True,
            )
        bcs.append(bc)
    # wave 3: ln(d)
    for bg in range(NG):
        lnd = ap.tile([128, 394], F32, tag=f"ln{bg}", name=f"lnd{h}_{bg}",
                      bufs=1)
        nc.scalar.activation(lnd[:, :], bcs[bg][:, :], ln)
        lnds.append(lnd)
    # wave 4: rc = exp(-ln d)
    for bg in range(NG):
        rc = ap.tile([128, 394], BF16, tag=f"rc{bg}", name=f"rc{h}_{bg}")
        nc.scalar.activation(rc[:, :], lnds[bg][:, :], exp, scale=-1.0)
        rcs.append(rc)
    # wave 5: cls, v-transpose, PV, normalize, hswish
    for bg in range(NG):
        b0 = 2 * bg
        E, rc = Es[bg], rcs[bg]
        for j in range(2):
            b = b0 + j
            # cls column: acc += E[:, q=0] * rc[:, q=0]  (per-partition scalar)
            nc.vector.scalar_tensor_tensor(
                out=acc_lo[0:128, b:b + 1], in0=E[0:128, j * 512:j * 512 + 1],
                scalar=rc[0:128, j * N:j * N + 1], in1=acc_lo[0:128, b:b + 1],
                op0=AX.mult, op1=AX.add,
            )
            nc.vector.scalar_tensor_tensor(
                out=acc_hi[0:69, b:b + 1],
                in0=E[0:69, j * 512 + 256:j * 512 + 257],
                scalar=rc[0:69, j * N:j * N + 1], in1=acc_hi[0:69, b:b + 1],
                op0=AX.mult, op1=AX.add,
            )
        vt = pB.tile([128, 512], BF16, tag="vt", name=f"vt{h}_{bg}")
        for j in range(2):
            c0 = (b0 + j) * N
            nc.tensor.transpose(
                vt[0:128, j * 256:j * 256 + 128], vt_src[:, c0:c0 + 128],
                ident_bf[:, :],
            )
            nc.tensor.transpose(
                vt[0:69, j * 256 + 128:j * 256 + 256],
                vt_src[:, c0 + 128:c0 + N], ident_bf[:, :],
            )
        vn = ap.tile([128, 512], BF16, tag="vn", name=f"vn{h}_{bg}")
        nc.vector.tensor_copy(vn[:, :], vt[:, :])
        ot = pO.tile([128, 394], F32, tag="ot", name=f"ot{h}_{bg}")
        for j in range(2):
            nc.tensor.matmul(
                ot[:, j * N:(j + 1) * N],
                lhsT=vn[0:128, j * 256:j * 256 + 128],
                rhs=E[0:128, j * 512:j * 512 + N],
                start=True, stop=False,
            )
            nc.tensor.matmul(
                ot[:, j * N:(j + 1) * N],
                lhsT=vn[0:69, j * 256 + 128:j * 256 + 256],
                rhs=E[0:69, j * 512 + 256:j * 512 + 256 + N],
                start=False, stop=True,
            )
        # normalize + hswish: hsw = on * (relu(on+3) - relu(on-3)),
        # with on = out_raw * (1/d); the /6 is folded into proj_w.
        on = ap.tile([128, 394], F32, tag="on", name=f"on{h}_{bg}")
        nc.vector.tensor_mul(on[:, :], ot[:, :], rc[:, :])
        a = ap.tile([128, 394], F32, tag="ha", name=f"ha{h}_{bg}")
        b2 = ap.tile([128, 394], F32, tag="hb", name=f"hb{h}_{bg}")
        nc.any.tensor_scalar(out=a[:, :], in0=on[:, :], scalar1=3.0,
                             scalar2=0.0, op0=AX.add, op1=AX.max)
        nc.any.tensor_scalar(out=b2[:, :], in0=on[:, :], scalar1=-3.0,
                             scalar2=0.0, op0=AX.add, op1=AX.max)
        nc.any.tensor_sub(a[:, :], a[:, :], b2[:, :])
        nc.any.tensor_mul(hsw_h[:, b0 * N:b0 * N + 2 * N], on[:, :], a[:, :])


# ----------------------------------------------------------------------------
# host side
# ----------------------------------------------------------------------------

_CACHE = {}


def _build():
    if "nc" not in _CACHE:
        nc = bacc.Bacc("TRN2", target_bir_lowering=False, debug=False,
                       num_devices=NC_CORES)
        _emit(nc)
        nc.compile()
        _CACHE["nc"] = nc
    return _CACHE["nc"]


def _feature_perm():
    """Permutation of qkv output features into [Q | K | V] blocks so each
    head's q and k live at the same partition offset of their SBUF tiles."""
    perm = np.empty(HQKV, dtype=np.int64)
    pos = 0
    for blk, width in ((0, KD), (KD, KD), (2 * KD, D)):  # q, k, v offsets
        for h in range(H):
            perm[pos:pos + width] = h * (2 * KD + D) + blk + np.arange(width)
            pos += width
    return perm


def _prep_shared(qkv_w, qkv_g, qkv_b, proj_w, proj_g, proj_b):
    perm = _feature_perm()
    wq = qkv_w[perm]                      # [3072, 768]
    g1 = qkv_g[perm]
    b1 = qkv_b[perm]
    wqT = np.ascontiguousarray(wq.T, dtype=np.float32)          # [768, 3072]
    wpT = np.ascontiguousarray(proj_w.T / 6.0, dtype=np.float32)  # [1536, 768]
    gb1 = np.concatenate(
        [g1.reshape(NFT, 128).T, b1.reshape(NFT, 128).T], axis=1
    ).astype(np.float32)                  # [128, 48]
    gb2 = np.concatenate(
        [proj_g.reshape(NFP, 128).T, proj_b.reshape(NFP, 128).T], axis=1
    ).astype(np.float32)                  # [128, 12]
    ident = np.eye(128, dtype=np.float32)
    return wqT, wpT, gb1, gb2, ident


def run(x, qkv_w, qkv_g, qkv_b, proj_w, proj_g, proj_b, trace=False,
        trace_kwargs=None):
    nc = _build()
    wqT, wpT, gb1, gb2, ident = _prep_shared(
        qkv_w, qkv_g, qkv_b, proj_w, proj_g, proj_b)
    in_maps = []
    for i in range(NC_CORES):
        xs = x[i * BL:(i + 1) * BL].reshape(TL, DIM)
        in_maps.append({
            "xT": np.ascontiguousarray(xs.T, dtype=np.float32),
            "wqT": wqT, "wpT": wpT, "gb1": gb1, "gb2": gb2, "ident": ident,
        })
    kw = {}
    if trace:
        kw["trace"] = True
        if trace_kwargs:
            kw.update(trace_kwargs)
    res = run_bass_kernel_spmd(nc, in_maps, list(range(NC_CORES)), **kw)
    y = np.concatenate(
        [r["yT"].T.reshape(BL, N, DIM) for r in res.results], axis=0)
    cls = np.concatenate(
        [r["cls"].reshape(BL, 200)[:, 1:N] for r in res.results], axis=0)
    return y, cls, res


def kernel(x, qkv_w, qkv_g, qkv_b, proj_w, proj_g, proj_b):
    y, cls, _ = run(x, qkv_w, qkv_g, qkv_b, proj_w, proj_g, proj_b)
    return y, cls
